# revision 18
# baseline (speedup 1.0000x reference)
"""Trainium2 Bass kernel for nn_AttnReweight (superpixel-reweighted attention).

Math (per batch b, head hd, pixel (h,w), key k in a 7x7 window):
    w[b,h,w,k] = sum_{s in 3x3 superpixel nbhd} Pi[b,h,w,s] * Pj[b,s,h,w,k]
    out = (w * exp(attn)) / (eps + sum_k w * exp(attn))
(The reference's max-shift cancels in the ratio; attn ~ N(0,1) so exp() is
safe in fp32 without it.)

Sharding: 8 cores = 2 batches x 4 row-bands of 64 rows. Each core gets
  - its attn shard, pre-swizzled to the on-chip (tile, head, block, pixel)
    layout so loads/stores are two maximal contiguous DMAs per (tile, head)
  - a "slab" shard: for each of its 70 rows (64 + 3 halo each side, rows
    clamped at the image border) the 5 superpixel-table rows that any query
    window positioned at that row can touch, zero-masked where the plane
    index falls outside the 32x32 superpixel grid.
All remaining work is on-device and identical on every core (SPMD):
per-pixel 5x5 window extraction, per-block (8x8-pixel) region tiles,
the 9-term superpixel einsum, exp/normalize, and the output writeback.
"""

import sys

sys.path.insert(0, "/opt/trn_rl_repo")

import numpy as np

import concourse.bass as bass
import concourse.tile as tile
from concourse import bacc, mybir
from contextlib import ExitStack

F32 = mybir.dt.float32
BF16 = mybir.dt.bfloat16

# problem geometry (hardcoded per the harness contract)
B, HD, H, W, K = 2, 4, 256, 256, 49
SH = SW = 32
N_CORES = 8
BAND = 64          # pixel rows per core
HALO = 3
NROW = BAND + 2 * HALO          # 70 A rows per core
NT = 2                          # tiles per core (block-row halves)
HBT = 4                         # block-rows per tile
NBW = 32                        # block-cols
P = HBT * NBW                   # 128 partitions (blocks) per tile
NQ = 14 * 14                    # region pixels per block
NI = 64                         # pixels per block
NK = 49
NS = 9
APAD = 75                       # 3 pixels * 25 on each w side
AFS = APAD + 256 * 25 + APAD    # A free size (w-major, 25-patch inner)
G25FS = NQ * 25                 # 4900
NQ16 = 14 * 16                  # padded region row pitch
G9FS = NS * NQ16                # 2016
EFS = NI * NK                   # 3136 (compact i,k)
EFSP = NI * 56                  # 3584 (k padded to 56 for alignment)
WC = 32                         # slab w-chunk
SLABPAD = 64
SLABFS = WC * 160 + 2 * SLABPAD


def APx(t, off, dims):
    return bass.AP(t.tensor, off, [list(d) for d in dims])


def build_graph():
    nc = bacc.Bacc("TRN2", target_bir_lowering=False, debug=False,
                   num_devices=N_CORES)
    attn_d = nc.dram_tensor("attn", [NT, HD, P, EFS], F32, kind="ExternalInput").ap()
    slab_d = nc.dram_tensor("slab", [NROW, W, 5, SW], F32, kind="ExternalInput").ap()
    out_d = nc.dram_tensor("out", [NT, HD, P, EFS], F32, kind="ExternalOutput").ap()

    mult, add = mybir.AluOpType.mult, mybir.AluOpType.add

    with tile.TileContext(nc) as tc, ExitStack() as ctx:
        slab_pool = ctx.enter_context(tc.tile_pool(name="slab", bufs=2))
        a_pool = ctx.enter_context(tc.tile_pool(name="apool", bufs=1))
        g25_pool = ctx.enter_context(tc.tile_pool(name="g25", bufs=1))
        g9_pool = ctx.enter_context(tc.tile_pool(name="g9", bufs=2))
        pix_pool = ctx.enter_context(tc.tile_pool(name="pix", bufs=1))
        e_pool = ctx.enter_context(tc.tile_pool(name="epool", bufs=1))
        eb_pool = ctx.enter_context(tc.tile_pool(name="ebpool", bufs=2))
        y_pool = ctx.enter_context(tc.tile_pool(name="ypool", bufs=2))
        w_pool = ctx.enter_context(tc.tile_pool(name="wpool", bufs=2))
        tmp_pool = ctx.enter_context(tc.tile_pool(name="tmp", bufs=2))
        wg_pool = ctx.enter_context(tc.tile_pool(name="wgpool", bufs=1))
        s_pool = ctx.enter_context(tc.tile_pool(name="spool", bufs=4))
        d_pool = ctx.enter_context(tc.tile_pool(name="dstage", bufs=1, space="DRAM"))

        A = a_pool.tile([NROW, AFS], BF16)
        Ad = d_pool.tile([NROW, AFS], BF16)
        # zero the w-padding columns once (read by the full-width G25 DMA)
        nc.vector.memset(APx(A, 0, [[AFS, NROW], [1, APAD]]), 0.0)
        nc.vector.memset(APx(A, APAD + 256 * 25, [[AFS, NROW], [1, APAD]]), 0.0)

        # ---- stage 1: slab load + per-pixel 5x5 window extraction into A
        # A[r, 75 + w*25 + th*5 + tw] = slab[r, w, th, (w//8) + tw - 2]
        for c in range(W // WC):
            SB = slab_pool.tile([NROW, SLABFS], F32)
            nc.vector.memset(APx(SB, 0, [[SLABFS, NROW], [1, SLABPAD]]), 0.0)
            nc.vector.memset(
                APx(SB, SLABPAD + WC * 160, [[SLABFS, NROW], [1, SLABPAD]]), 0.0)
            nc.sync.dma_start(
                APx(SB, SLABPAD, [[SLABFS, NROW], [1, WC * 160]]),
                APx(slab_d, c * WC * 160, [[W * 160, NROW], [1, WC * 160]]),
            )
            nwb = WC // 8
            src = APx(SB, SLABPAD + (c * nwb) - 2,
                      [[SLABFS, NROW], [8 * 160 + 1, nwb], [160, 8], [32, 5], [1, 5]])
            dst = APx(A, APAD + c * WC * 25,
                      [[AFS, NROW], [200, nwb], [25, 8], [5, 5], [1, 5]])
            nc.vector.tensor_copy(dst, src)

        # zero window columns whose superpixel column falls outside [0,32)
        for w0, nw, tc0, ntc in ((0, 8, 0, 2), (8, 8, 0, 1),
                                 (240, 8, 4, 1), (248, 8, 3, 2)):
            nc.vector.memset(
                APx(A, APAD + w0 * 25 + tc0,
                    [[AFS, NROW], [25, nw], [5, 5], [1, ntc]]), 0.0)
        # fill the w-padding with the border pixel's patch, re-expressed in
        # the out-of-range region position's frame (clipped key pixels)
        nc.vector.tensor_copy(
            APx(A, 0 * 25 + 2, [[AFS, NROW], [25, 3], [5, 5], [1, 3]]),
            APx(A, APAD + 0 * 25 + 1, [[AFS, NROW], [0, 3], [5, 5], [1, 3]]),
        )
        nc.vector.tensor_copy(
            APx(A, APAD + 256 * 25 + 0, [[AFS, NROW], [25, 3], [5, 5], [1, 3]]),
            APx(A, APAD + 255 * 25 + 1, [[AFS, NROW], [0, 3], [5, 5], [1, 3]]),
        )
        # stage A to DRAM (SBUF APs cannot express the partition-crossing
        # A -> G25 rearrange on both sides; DRAM APs are flat)
        nc.sync.dma_start(Ad[:], A[:])

        # ---- per-tile processing
        for T in range(NT):
            # G25[p = hbl*32+wb, (qh*14+qw)*25 + t] = A[32T+8hbl+qh, w=8wb+qw-3, t]
            G25 = g25_pool.tile([P, G25FS], BF16)
            for hbl in range(HBT):
                nc.sync.dma_start(
                    APx(G25, hbl * 32 * G25FS,
                        [[G25FS, NBW], [14 * 25, 14], [1, 350]]),
                    APx(Ad, (32 * T + 8 * hbl) * AFS + APAD - 3 * 25,
                        [[200, NBW], [AFS, 14], [1, 350]]),
                )

            # ---- G9: rectangularize per (s, dd); ACT + GpSimd do the copies
            G9 = g9_pool.tile([P, G9FS], BF16)
            nc.gpsimd.memset(
                APx(G9, 14, [[G9FS, P], [16, NS * 14], [1, 2]]), 0.0)
            engs = [nc.scalar, nc.gpsimd, nc.vector]
            ci = 0
            for si in range(NS):
                dh, dw = si // 3 - 1, si % 3 - 1
                for ddh in (-1, 0, 1):
                    for ddw in (-1, 0, 1):
                        qh0, nqh = {(-1): (0, 3), 0: (3, 8), 1: (11, 3)}[ddh]
                        qw0, nqw = {(-1): (0, 3), 0: (3, 8), 1: (11, 3)}[ddw]
                        tcol = (dh - ddh + 2) * 5 + (dw - ddw + 2)
                        src = APx(G25, (qh0 * 14 + qw0) * 25 + tcol,
                                  [[G25FS, P], [14 * 25, nqh], [25, nqw]])
                        dst = APx(G9, si * NQ16 + qh0 * 16 + qw0,
                                  [[G9FS, P], [16, nqh], [1, nqw]])
                        eng = engs[ci % 3]
                        ci += 1
                        if eng is nc.scalar:
                            eng.copy(dst, src)
                        else:
                            eng.tensor_copy(dst, src)

            # ---- einsum: W[p, i, kpad56] = sum_s Pi_s * Pj_s
            # Pi is pre-expanded per term (PiX[s][p, (ih, iw, kw7)]) so the
            # kh-peeled multiplies run with step-1 operands (2x bf16 mode).
            # layouts: W/tmp/Y rows are (i, kh, kw) at i*56 + kh*8 + kw with
            # pad column kw=7; the (i,kh) pair merges into one stride-8 dim
            # of 448 (m = 7i + kh), giving 2-dim non-pad views.
            Wv = w_pool.tile([P, EFSP], BF16)
            Wg = wg_pool.tile([P, EFSP], BF16)
            PiX = pix_pool.tile([P, NS * 512], BF16)
            nc.vector.memset(APx(PiX, 7, [[NS * 512, P], [8, NS * 64]]), 0.0)
            for si in range(NS):
                nc.vector.tensor_copy(
                    APx(PiX, si * 512, [[NS * 512, P], [64, 8], [8, 8], [1, 7]]),
                    APx(G9, si * NQ16 + 51, [[G9FS, P], [16, 8], [1, 8], [0, 7]]),
                )

            def term(eng, si, dst):
                for kh in range(7):
                    eng.tensor_tensor(
                        APx(dst, kh * 8, [[EFSP, P], [448, 8], [56, 8], [1, 8]]),
                        APx(PiX, si * 512, [[NS * 512, P], [64, 8], [8, 8], [1, 8]]),
                        APx(G9, si * NQ16 + kh * 16,
                            [[G9FS, P], [16, 8], [1, 8], [1, 8]]),
                        op=mult)

            def nopad(t):
                return APx(t, 0, [[EFSP, P], [1, EFSP]])

            term(nc.vector, 0, Wv)
            for si in (1, 2, 3, 4, 5):
                tmpD = tmp_pool.tile([P, EFSP], BF16, tag="tmpd")
                term(nc.vector, si, tmpD)
                nc.vector.tensor_tensor(nopad(Wv), nopad(Wv), nopad(tmpD), op=add)
            term(nc.gpsimd, 6, Wg)
            for si in (7, 8):
                tmpG = tmp_pool.tile([P, EFSP], BF16, tag="tmpg")
                term(nc.gpsimd, si, tmpG)
                nc.gpsimd.tensor_tensor(nopad(Wg), nopad(Wg), nopad(tmpG), op=add)
            nc.vector.tensor_tensor(nopad(Wv), nopad(Wv), nopad(Wg), op=add)

            # ---- per-head: attn -> exp -> y -> sum_k -> normalize -> out
            for hd in range(HD):
                E = e_pool.tile([P, EFS], F32)
                nc.scalar.dma_start(
                    E[:],
                    APx(attn_d, (T * HD + hd) * P * EFS, [[EFS, P], [1, EFS]]),
                )
                Eb = eb_pool.tile([P, EFS], BF16)
                nc.scalar.activation(Eb[:], E[:], mybir.ActivationFunctionType.Exp)
                Yp = y_pool.tile([P, EFSP], BF16)
                eng = nc.vector if hd < 3 else nc.gpsimd
                nc.vector.memset(APx(Yp, 7, [[EFSP, P], [8, 448]]), 0.0)
                eng.tensor_tensor(
                    APx(Yp, 0, [[EFSP, P], [8, 448], [1, 7]]),
                    APx(Eb, 0, [[EFS, P], [7, 448], [1, 7]]),
                    APx(Wv, 0, [[EFSP, P], [8, 448], [1, 7]]), op=mult)
                Ssum = s_pool.tile([P, NI], F32, tag="ssum")
                Rcp = s_pool.tile([P, NI], F32, tag="rcp")
                nc.vector.tensor_reduce(
                    Ssum[:], APx(Yp, 0, [[EFSP, P], [56, NI], [1, 56]]),
                    axis=mybir.AxisListType.X, op=add)
                nc.vector.tensor_scalar_add(Rcp[:], Ssum[:], 1e-15)
                nc.vector.reciprocal(Rcp[:], Rcp[:])
                # normalize, writing f32 compact into the (now free) E tile
                eng.tensor_tensor(
                    APx(E, 0, [[EFS, P], [49, 64], [7, 7], [1, 7]]),
                    APx(Yp, 0, [[EFSP, P], [56, 64], [8, 7], [1, 7]]),
                    APx(Rcp, 0, [[NI, P], [1, NI], [0, 7], [0, 7]]), op=mult)
                nc.sync.dma_start(
                    APx(out_d, (T * HD + hd) * P * EFS, [[EFS, P], [1, EFS]]),
                    E[:],
                )

    nc.compile()
    return nc


def shard_inputs(attn, sims):
    """Full inputs -> per-core in_maps (list of 8 dicts)."""
    attn = np.ascontiguousarray(attn, dtype=np.float32)
    sims = np.ascontiguousarray(sims, dtype=np.float32)
    in_maps = []
    th = np.arange(5)
    for c in range(N_CORES):
        b, j = divmod(c, 4)
        a = attn[b, :, 64 * j:64 * j + 64]            # (hd, 64, 256, 49)
        a = a.reshape(HD, NT, HBT, 8, NBW, 8, K)
        a = a.transpose(1, 0, 2, 4, 3, 5, 6)          # T, hd, hbl, wb, ih, iw, k
        attn_shard = np.ascontiguousarray(a.reshape(NT, HD, P, EFS))
        gpos = np.arange(64 * j - HALO, 64 * j + BAND + HALO)
        gval = np.clip(gpos, 0, H - 1)
        rows = sims[b, gval]                          # (70, 256, 32, 32)
        sh = (gpos[:, None] // 8) + th[None, :] - 2   # (70, 5)
        valid = (sh >= 0) & (sh < SH)
        shc = np.clip(sh, 0, SH - 1)
        slab = np.take_along_axis(rows, shc[:, None, :, None], axis=2)
        slab = np.where(valid[:, None, :, None], slab, np.float32(0.0))
        in_maps.append({"attn": attn_shard,
                        "slab": np.ascontiguousarray(slab, dtype=np.float32)})
    return in_maps


def unshard_output(results):
    out = np.empty((B, HD, H, W, K), dtype=np.float32)
    for c in range(N_CORES):
        b, j = divmod(c, 4)
        o = results[c]["out"].reshape(NT, HD, HBT, NBW, 8, 8, K)
        o = o.transpose(1, 0, 2, 4, 3, 5, 6)          # hd, T, hbl, ih, wb, iw, k
        out[b, :, 64 * j:64 * j + 64] = o.reshape(HD, BAND, W, K)
    return out


_NC_CACHE = {}


def kernel(attn, sims):
    from concourse.bass_utils import run_bass_kernel_spmd
    if "nc" not in _NC_CACHE:
        _NC_CACHE["nc"] = build_graph()
    nc = _NC_CACHE["nc"]
    in_maps = shard_inputs(attn, sims)
    res = run_bass_kernel_spmd(nc, in_maps, core_ids=list(range(N_CORES)))
    return unshard_output(res.results)


# revision 19
# speedup vs baseline: 1.1703x; 1.1703x over previous
"""Trainium2 Bass kernel for nn_AttnReweight (superpixel-reweighted attention).

Math (per batch b, head hd, pixel (h,w), key k in a 7x7 window):
    w[b,h,w,k] = sum_{s in 3x3 superpixel nbhd} Pi[b,h,w,s] * Pj[b,s,h,w,k]
    out = (w * exp(attn)) / (eps + sum_k w * exp(attn))
(The reference's max-shift cancels in the ratio; attn ~ N(0,1) so exp() is
safe in fp32 without it.)

Sharding: 8 cores = 2 batches x 4 row-bands of 64 rows. Each core gets
  - its attn shard, pre-swizzled to the on-chip (tile, head, block, pixel)
    layout so loads/stores are two maximal contiguous DMAs per (tile, head)
  - a "slab" shard: for each of its 70 rows (64 + 3 halo each side, rows
    clamped at the image border) the 5 superpixel-table rows that any query
    window positioned at that row can touch, zero-masked where the plane
    index falls outside the 32x32 superpixel grid.
All remaining work is on-device and identical on every core (SPMD):
per-pixel 5x5 window extraction, per-block (8x8-pixel) region tiles,
the 9-term superpixel einsum, exp/normalize, and the output writeback.
"""

import sys

sys.path.insert(0, "/opt/trn_rl_repo")

import numpy as np

import concourse.bass as bass
import concourse.tile as tile
from concourse import bacc, mybir
from contextlib import ExitStack

F32 = mybir.dt.float32
BF16 = mybir.dt.bfloat16

# problem geometry (hardcoded per the harness contract)
B, HD, H, W, K = 2, 4, 256, 256, 49
SH = SW = 32
N_CORES = 8
BAND = 64          # pixel rows per core
HALO = 3
NROW = BAND + 2 * HALO          # 70 A rows per core
NT = 2                          # tiles per core (block-row halves)
HBT = 4                         # block-rows per tile
NBW = 32                        # block-cols
P = HBT * NBW                   # 128 partitions (blocks) per tile
NQ = 14 * 14                    # region pixels per block
NI = 64                         # pixels per block
NK = 49
NS = 9
APAD = 75                       # 3 pixels * 25 on each w side
AFS = APAD + 256 * 25 + APAD    # A free size (w-major, 25-patch inner)
G25FS = NQ * 25                 # 4900
G9FS = NS * NQ                  # 1764
EFS = NI * NK                   # 3136 (compact i,k)
EFSP = NI * 56                  # 3584 (k padded to 56 for alignment)
WC = 32                         # slab w-chunk
SLABPAD = 64
SLABFS = WC * 160 + 2 * SLABPAD


def APx(t, off, dims):
    return bass.AP(t.tensor, off, [list(d) for d in dims])


def build_graph():
    nc = bacc.Bacc("TRN2", target_bir_lowering=False, debug=False,
                   num_devices=N_CORES)
    attn_d = nc.dram_tensor("attn", [NT, HD, P, EFS], F32, kind="ExternalInput").ap()
    slab_d = nc.dram_tensor("slab", [NROW, W, 5, SW], F32, kind="ExternalInput").ap()
    out_d = nc.dram_tensor("out", [NT, HD, P, EFS], F32, kind="ExternalOutput").ap()

    mult, add = mybir.AluOpType.mult, mybir.AluOpType.add

    with tile.TileContext(nc) as tc, ExitStack() as ctx:
        slab_pool = ctx.enter_context(tc.tile_pool(name="slab", bufs=2))
        a_pool = ctx.enter_context(tc.tile_pool(name="apool", bufs=1))
        g25_pool = ctx.enter_context(tc.tile_pool(name="g25", bufs=2))
        g9_pool = ctx.enter_context(tc.tile_pool(name="g9", bufs=2))
        pix_pool = ctx.enter_context(tc.tile_pool(name="pix", bufs=2))
        e_pool = ctx.enter_context(tc.tile_pool(name="epool", bufs=2))
        eb_pool = ctx.enter_context(tc.tile_pool(name="ebpool", bufs=2))
        y_pool = ctx.enter_context(tc.tile_pool(name="ypool", bufs=2))
        w_pool = ctx.enter_context(tc.tile_pool(name="wpool", bufs=2))
        tmp_pool = ctx.enter_context(tc.tile_pool(name="tmp", bufs=2))
        wg_pool = ctx.enter_context(tc.tile_pool(name="wgpool", bufs=2))
        s_pool = ctx.enter_context(tc.tile_pool(name="spool", bufs=4))
        d_pool = ctx.enter_context(tc.tile_pool(name="dstage", bufs=1, space="DRAM"))

        A = a_pool.tile([NROW, AFS], BF16)
        Ad = d_pool.tile([NROW, AFS], BF16)
        # zero the w-padding columns once (read by the full-width G25 DMA)
        nc.vector.memset(APx(A, 0, [[AFS, NROW], [1, APAD]]), 0.0)
        nc.vector.memset(APx(A, APAD + 256 * 25, [[AFS, NROW], [1, APAD]]), 0.0)

        # ---- stage 1: slab load + per-pixel 5x5 window extraction into A
        # A[r, 75 + w*25 + th*5 + tw] = slab[r, w, th, (w//8) + tw - 2]
        for c in range(W // WC):
            SB = slab_pool.tile([NROW, SLABFS], F32)
            nc.vector.memset(APx(SB, 0, [[SLABFS, NROW], [1, SLABPAD]]), 0.0)
            nc.vector.memset(
                APx(SB, SLABPAD + WC * 160, [[SLABFS, NROW], [1, SLABPAD]]), 0.0)
            nc.sync.dma_start(
                APx(SB, SLABPAD, [[SLABFS, NROW], [1, WC * 160]]),
                APx(slab_d, c * WC * 160, [[W * 160, NROW], [1, WC * 160]]),
            )
            nwb = WC // 8
            src = APx(SB, SLABPAD + (c * nwb) - 2,
                      [[SLABFS, NROW], [8 * 160 + 1, nwb], [160, 8], [32, 5], [1, 5]])
            dst = APx(A, APAD + c * WC * 25,
                      [[AFS, NROW], [200, nwb], [25, 8], [5, 5], [1, 5]])
            nc.vector.tensor_copy(dst, src)

        # zero window columns whose superpixel column falls outside [0,32)
        for w0, nw, tc0, ntc in ((0, 8, 0, 2), (8, 8, 0, 1),
                                 (240, 8, 4, 1), (248, 8, 3, 2)):
            nc.vector.memset(
                APx(A, APAD + w0 * 25 + tc0,
                    [[AFS, NROW], [25, nw], [5, 5], [1, ntc]]), 0.0)
        # fill the w-padding with the border pixel's patch, re-expressed in
        # the out-of-range region position's frame (clipped key pixels)
        nc.vector.tensor_copy(
            APx(A, 0 * 25 + 2, [[AFS, NROW], [25, 3], [5, 5], [1, 3]]),
            APx(A, APAD + 0 * 25 + 1, [[AFS, NROW], [0, 3], [5, 5], [1, 3]]),
        )
        nc.vector.tensor_copy(
            APx(A, APAD + 256 * 25 + 0, [[AFS, NROW], [25, 3], [5, 5], [1, 3]]),
            APx(A, APAD + 255 * 25 + 1, [[AFS, NROW], [0, 3], [5, 5], [1, 3]]),
        )
        # stage A to DRAM (SBUF APs cannot express the partition-crossing
        # A -> G25 rearrange on both sides; DRAM APs are flat)
        nc.sync.dma_start(Ad[:], A[:])

        # ---- per-tile processing
        for T in range(NT):
            # G25[p = hbl*32+wb, (qh*14+qw)*25 + t] = A[32T+8hbl+qh, w=8wb+qw-3, t]
            G25 = g25_pool.tile([P, G25FS], BF16)
            for hbl in range(HBT):
                nc.sync.dma_start(
                    APx(G25, hbl * 32 * G25FS,
                        [[G25FS, NBW], [14 * 25, 14], [1, 350]]),
                    APx(Ad, (32 * T + 8 * hbl) * AFS + APAD - 3 * 25,
                        [[200, NBW], [AFS, 14], [1, 350]]),
                )

            # ---- G9: rectangularize per (s, dd); ACT + GpSimd do the copies
            G9 = g9_pool.tile([P, G9FS], BF16)
            engs = [nc.scalar, nc.gpsimd, nc.vector]
            ci = 0
            for si in range(NS):
                dh, dw = si // 3 - 1, si % 3 - 1
                for ddh in (-1, 0, 1):
                    for ddw in (-1, 0, 1):
                        qh0, nqh = {(-1): (0, 3), 0: (3, 8), 1: (11, 3)}[ddh]
                        qw0, nqw = {(-1): (0, 3), 0: (3, 8), 1: (11, 3)}[ddw]
                        tcol = (dh - ddh + 2) * 5 + (dw - ddw + 2)
                        src = APx(G25, (qh0 * 14 + qw0) * 25 + tcol,
                                  [[G25FS, P], [14 * 25, nqh], [25, nqw]])
                        dst = APx(G9, si * NQ + qh0 * 14 + qw0,
                                  [[G9FS, P], [14, nqh], [1, nqw]])
                        eng = engs[ci % 3]
                        ci += 1
                        if eng is nc.scalar:
                            eng.copy(dst, src)
                        else:
                            eng.tensor_copy(dst, src)

            # ---- einsum: W[p, i, kpad56] = sum_s Pi_s * Pj_s
            # Pi is pre-expanded per term (PiX[s][p, (ih, iw, kw7)]) so the
            # kh-peeled multiplies run with step-1 operands (2x bf16 mode).
            # layouts: W/tmp/Y rows are (i, kh, kw) at i*56 + kh*8 + kw with
            # pad column kw=7; the (i,kh) pair merges into one stride-8 dim
            # of 448 (m = 7i + kh), giving 2-dim non-pad views.
            Wv = w_pool.tile([P, EFSP], BF16)
            Wg = wg_pool.tile([P, EFSP], BF16)
            PiX = pix_pool.tile([P, NS * 512], BF16)
            for si in range(NS):
                nc.vector.tensor_copy(
                    APx(PiX, si * 512, [[NS * 512, P], [64, 8], [8, 8], [1, 7]]),
                    APx(G9, si * NQ + 45, [[G9FS, P], [14, 8], [1, 8], [0, 7]]),
                )

            def term(eng, si, dst):
                for kh in range(7):
                    eng.tensor_tensor(
                        APx(dst, kh * 8, [[EFSP, P], [448, 8], [56, 8], [1, 7]]),
                        APx(PiX, si * 512, [[NS * 512, P], [64, 8], [8, 8], [1, 7]]),
                        APx(G9, si * NQ + kh * 14,
                            [[G9FS, P], [14, 8], [1, 8], [1, 7]]),
                        op=mult)

            def nopad(t):
                return APx(t, 0, [[EFSP, P], [8, 448], [1, 7]])

            # zero the k-padding columns (so pads contribute 0 downstream)
            nc.vector.memset(APx(Wv, 7, [[EFSP, P], [8, 448]]), 0.0)
            term(nc.vector, 0, Wv)
            for si in (1, 2, 3, 4, 5):
                tmpD = tmp_pool.tile([P, EFSP], BF16, tag="tmpd")
                term(nc.vector, si, tmpD)
                nc.vector.tensor_tensor(nopad(Wv), nopad(Wv), nopad(tmpD), op=add)
            term(nc.gpsimd, 6, Wg)
            for si in (7, 8):
                tmpG = tmp_pool.tile([P, EFSP], BF16, tag="tmpg")
                term(nc.gpsimd, si, tmpG)
                nc.gpsimd.tensor_tensor(nopad(Wg), nopad(Wg), nopad(tmpG), op=add)
            nc.vector.tensor_tensor(nopad(Wv), nopad(Wv), nopad(Wg), op=add)

            # ---- per-head: attn -> exp -> y -> sum_k -> normalize -> out
            for hd in range(HD):
                E = e_pool.tile([P, EFS], F32)
                nc.scalar.dma_start(
                    E[:],
                    APx(attn_d, (T * HD + hd) * P * EFS, [[EFS, P], [1, EFS]]),
                )
                Eb = eb_pool.tile([P, EFS], BF16)
                nc.scalar.activation(Eb[:], E[:], mybir.ActivationFunctionType.Exp)
                Yp = y_pool.tile([P, EFSP], BF16)
                eng = nc.vector if hd < 3 else nc.gpsimd
                nc.vector.memset(APx(Yp, 7, [[EFSP, P], [8, 448]]), 0.0)
                eng.tensor_tensor(
                    APx(Yp, 0, [[EFSP, P], [8, 448], [1, 7]]),
                    APx(Eb, 0, [[EFS, P], [7, 448], [1, 7]]),
                    APx(Wv, 0, [[EFSP, P], [8, 448], [1, 7]]), op=mult)
                Ssum = s_pool.tile([P, NI], F32, tag="ssum")
                Rcp = s_pool.tile([P, NI], F32, tag="rcp")
                nc.vector.tensor_reduce(
                    Ssum[:], APx(Yp, 0, [[EFSP, P], [56, NI], [1, 56]]),
                    axis=mybir.AxisListType.X, op=add)
                nc.vector.tensor_scalar_add(Rcp[:], Ssum[:], 1e-15)
                nc.vector.reciprocal(Rcp[:], Rcp[:])
                # normalize, writing f32 compact into the (now free) E tile
                eng.tensor_tensor(
                    APx(E, 0, [[EFS, P], [49, 64], [7, 7], [1, 7]]),
                    APx(Yp, 0, [[EFSP, P], [56, 64], [8, 7], [1, 7]]),
                    APx(Rcp, 0, [[NI, P], [1, NI], [0, 7], [0, 7]]), op=mult)
                nc.sync.dma_start(
                    APx(out_d, (T * HD + hd) * P * EFS, [[EFS, P], [1, EFS]]),
                    E[:],
                )

    nc.compile()
    return nc


def shard_inputs(attn, sims):
    """Full inputs -> per-core in_maps (list of 8 dicts)."""
    attn = np.ascontiguousarray(attn, dtype=np.float32)
    sims = np.ascontiguousarray(sims, dtype=np.float32)
    in_maps = []
    th = np.arange(5)
    for c in range(N_CORES):
        b, j = divmod(c, 4)
        a = attn[b, :, 64 * j:64 * j + 64]            # (hd, 64, 256, 49)
        a = a.reshape(HD, NT, HBT, 8, NBW, 8, K)
        a = a.transpose(1, 0, 2, 4, 3, 5, 6)          # T, hd, hbl, wb, ih, iw, k
        attn_shard = np.ascontiguousarray(a.reshape(NT, HD, P, EFS))
        gpos = np.arange(64 * j - HALO, 64 * j + BAND + HALO)
        gval = np.clip(gpos, 0, H - 1)
        rows = sims[b, gval]                          # (70, 256, 32, 32)
        sh = (gpos[:, None] // 8) + th[None, :] - 2   # (70, 5)
        valid = (sh >= 0) & (sh < SH)
        shc = np.clip(sh, 0, SH - 1)
        slab = np.take_along_axis(rows, shc[:, None, :, None], axis=2)
        slab = np.where(valid[:, None, :, None], slab, np.float32(0.0))
        in_maps.append({"attn": attn_shard,
                        "slab": np.ascontiguousarray(slab, dtype=np.float32)})
    return in_maps


def unshard_output(results):
    out = np.empty((B, HD, H, W, K), dtype=np.float32)
    for c in range(N_CORES):
        b, j = divmod(c, 4)
        o = results[c]["out"].reshape(NT, HD, HBT, NBW, 8, 8, K)
        o = o.transpose(1, 0, 2, 4, 3, 5, 6)          # hd, T, hbl, ih, wb, iw, k
        out[b, :, 64 * j:64 * j + 64] = o.reshape(HD, BAND, W, K)
    return out


_NC_CACHE = {}


def kernel(attn, sims):
    from concourse.bass_utils import run_bass_kernel_spmd
    if "nc" not in _NC_CACHE:
        _NC_CACHE["nc"] = build_graph()
    nc = _NC_CACHE["nc"]
    in_maps = shard_inputs(attn, sims)
    res = run_bass_kernel_spmd(nc, in_maps, core_ids=list(range(N_CORES)))
    return unshard_output(res.results)


# revision 21
# speedup vs baseline: 1.2104x; 1.0343x over previous
"""Trainium2 Bass kernel for nn_AttnReweight (superpixel-reweighted attention).

Math (per batch b, head hd, pixel (h,w), key k in a 7x7 window):
    w[b,h,w,k] = sum_{s in 3x3 superpixel nbhd} Pi[b,h,w,s] * Pj[b,s,h,w,k]
    out = (w * exp(attn)) / (eps + sum_k w * exp(attn))
(The reference's max-shift cancels in the ratio; attn ~ N(0,1) so exp() is
safe in fp32 without it.)

Sharding: 8 cores = 2 batches x 4 row-bands of 64 rows. Each core gets
  - its attn shard, pre-swizzled to the on-chip (tile, head, block, pixel)
    layout so loads/stores are two maximal contiguous DMAs per (tile, head)
  - a "slab" shard: for each of its 70 rows (64 + 3 halo each side, rows
    clamped at the image border) the 5 superpixel-table rows that any query
    window positioned at that row can touch, zero-masked where the plane
    index falls outside the 32x32 superpixel grid.
All remaining work is on-device and identical on every core (SPMD):
per-pixel 5x5 window extraction, per-block (8x8-pixel) region tiles,
the 9-term superpixel einsum, exp/normalize, and the output writeback.
"""

import sys

sys.path.insert(0, "/opt/trn_rl_repo")

import numpy as np

import concourse.bass as bass
import concourse.tile as tile
from concourse import bacc, mybir
from contextlib import ExitStack

F32 = mybir.dt.float32
BF16 = mybir.dt.bfloat16

# problem geometry (hardcoded per the harness contract)
B, HD, H, W, K = 2, 4, 256, 256, 49
SH = SW = 32
N_CORES = 8
BAND = 64          # pixel rows per core
HALO = 3
NROW = BAND + 2 * HALO          # 70 A rows per core
NT = 2                          # tiles per core (block-row halves)
HBT = 4                         # block-rows per tile
NBW = 32                        # block-cols
P = HBT * NBW                   # 128 partitions (blocks) per tile
NQ = 14 * 14                    # region pixels per block
NI = 64                         # pixels per block
NK = 49
NS = 9
APAD = 75                       # 3 pixels * 25 on each w side
AFS = APAD + 256 * 25 + APAD    # A free size (w-major, 25-patch inner)
G25FS = NQ * 25                 # 4900
NQ16 = 14 * 16                  # padded region row pitch
G9FS = NS * NQ16                # 2016
EFS = NI * NK                   # 3136 (compact i,k)
EFSP = NI * 56                  # 3584 (k padded to 56 for alignment)
WC = 32                         # slab w-chunk
SLABPAD = 64
SLABFS = WC * 160 + 2 * SLABPAD


def APx(t, off, dims):
    return bass.AP(t.tensor, off, [list(d) for d in dims])


def build_graph():
    nc = bacc.Bacc("TRN2", target_bir_lowering=False, debug=False,
                   num_devices=N_CORES)
    attn_d = nc.dram_tensor("attn", [NT, HD, P, EFS], F32, kind="ExternalInput").ap()
    slab_d = nc.dram_tensor("slab", [NROW, W, 5, SW], BF16, kind="ExternalInput").ap()
    out_d = nc.dram_tensor("out", [NT, HD, P, EFS], F32, kind="ExternalOutput").ap()

    mult, add = mybir.AluOpType.mult, mybir.AluOpType.add

    with tile.TileContext(nc) as tc, ExitStack() as ctx:
        slab_pool = ctx.enter_context(tc.tile_pool(name="slab", bufs=2))
        a_pool = ctx.enter_context(tc.tile_pool(name="apool", bufs=1))
        g25_pool = ctx.enter_context(tc.tile_pool(name="g25", bufs=2))
        g9_pool = ctx.enter_context(tc.tile_pool(name="g9", bufs=2))
        pix_pool = ctx.enter_context(tc.tile_pool(name="pix", bufs=2))
        e_pool = ctx.enter_context(tc.tile_pool(name="epool", bufs=2))
        eb_pool = ctx.enter_context(tc.tile_pool(name="ebpool", bufs=2))
        y_pool = ctx.enter_context(tc.tile_pool(name="ypool", bufs=2))
        w_pool = ctx.enter_context(tc.tile_pool(name="wpool", bufs=2))
        tmp_pool = ctx.enter_context(tc.tile_pool(name="tmp", bufs=2))
        wg_pool = ctx.enter_context(tc.tile_pool(name="wgpool", bufs=2))
        s_pool = ctx.enter_context(tc.tile_pool(name="spool", bufs=4))
        d_pool = ctx.enter_context(tc.tile_pool(name="dstage", bufs=1, space="DRAM"))

        A = a_pool.tile([NROW, AFS], BF16)
        Ad = d_pool.tile([NROW, AFS], BF16)
        # zero the w-padding columns once (read by the full-width G25 DMA)
        nc.vector.memset(APx(A, 0, [[AFS, NROW], [1, APAD]]), 0.0)
        nc.vector.memset(APx(A, APAD + 256 * 25, [[AFS, NROW], [1, APAD]]), 0.0)

        # ---- stage 1: slab load + per-pixel 5x5 window extraction into A
        # A[r, 75 + w*25 + th*5 + tw] = slab[r, w, th, (w//8) + tw - 2]
        for c in range(W // WC):
            SB = slab_pool.tile([NROW, SLABFS], BF16)
            nc.vector.memset(APx(SB, 0, [[SLABFS, NROW], [1, SLABPAD]]), 0.0)
            nc.vector.memset(
                APx(SB, SLABPAD + WC * 160, [[SLABFS, NROW], [1, SLABPAD]]), 0.0)
            nc.sync.dma_start(
                APx(SB, SLABPAD, [[SLABFS, NROW], [1, WC * 160]]),
                APx(slab_d, c * WC * 160, [[W * 160, NROW], [1, WC * 160]]),
            )
            nwb = WC // 8
            src = APx(SB, SLABPAD + (c * nwb) - 2,
                      [[SLABFS, NROW], [8 * 160 + 1, nwb], [160, 8], [32, 5], [1, 5]])
            dst = APx(A, APAD + c * WC * 25,
                      [[AFS, NROW], [200, nwb], [25, 8], [5, 5], [1, 5]])
            nc.vector.tensor_copy(dst, src)

        # zero window columns whose superpixel column falls outside [0,32)
        for w0, nw, tc0, ntc in ((0, 8, 0, 2), (8, 8, 0, 1),
                                 (240, 8, 4, 1), (248, 8, 3, 2)):
            nc.vector.memset(
                APx(A, APAD + w0 * 25 + tc0,
                    [[AFS, NROW], [25, nw], [5, 5], [1, ntc]]), 0.0)
        # fill the w-padding with the border pixel's patch, re-expressed in
        # the out-of-range region position's frame (clipped key pixels)
        nc.vector.tensor_copy(
            APx(A, 0 * 25 + 2, [[AFS, NROW], [25, 3], [5, 5], [1, 3]]),
            APx(A, APAD + 0 * 25 + 1, [[AFS, NROW], [0, 3], [5, 5], [1, 3]]),
        )
        nc.vector.tensor_copy(
            APx(A, APAD + 256 * 25 + 0, [[AFS, NROW], [25, 3], [5, 5], [1, 3]]),
            APx(A, APAD + 255 * 25 + 1, [[AFS, NROW], [0, 3], [5, 5], [1, 3]]),
        )
        # stage A to DRAM (SBUF APs cannot express the partition-crossing
        # A -> G25 rearrange on both sides; DRAM APs are flat)
        nc.sync.dma_start(Ad[:], A[:])

        # ---- per-tile processing
        for T in range(NT):
            # G25[p = hbl*32+wb, (qh*14+qw)*25 + t] = A[32T+8hbl+qh, w=8wb+qw-3, t]
            G25 = g25_pool.tile([P, G25FS], BF16)
            for hbl in range(HBT):
                nc.sync.dma_start(
                    APx(G25, hbl * 32 * G25FS,
                        [[G25FS, NBW], [14 * 25, 14], [1, 350]]),
                    APx(Ad, (32 * T + 8 * hbl) * AFS + APAD - 3 * 25,
                        [[200, NBW], [AFS, 14], [1, 350]]),
                )

            # ---- G9: rectangularize per (s, dd); ACT + GpSimd do the copies
            G9 = g9_pool.tile([P, G9FS], BF16)
            nc.gpsimd.memset(
                APx(G9, 14, [[G9FS, P], [16, NS * 14], [1, 2]]), 0.0)
            engs = [nc.scalar, nc.gpsimd]
            ci = 0
            for si in range(NS):
                dh, dw = si // 3 - 1, si % 3 - 1
                for ddh in (-1, 0, 1):
                    for ddw in (-1, 0, 1):
                        qh0, nqh = {(-1): (0, 3), 0: (3, 8), 1: (11, 3)}[ddh]
                        qw0, nqw = {(-1): (0, 3), 0: (3, 8), 1: (11, 3)}[ddw]
                        tcol = (dh - ddh + 2) * 5 + (dw - ddw + 2)
                        src = APx(G25, (qh0 * 14 + qw0) * 25 + tcol,
                                  [[G25FS, P], [14 * 25, nqh], [25, nqw]])
                        dst = APx(G9, si * NQ16 + qh0 * 16 + qw0,
                                  [[G9FS, P], [16, nqh], [1, nqw]])
                        eng = engs[ci % 2]
                        ci += 1
                        if eng is nc.scalar:
                            eng.copy(dst, src)
                        else:
                            eng.tensor_copy(dst, src)

            # ---- einsum: W[p, i, kpad56] = sum_s Pi_s * Pj_s
            # Pi is pre-expanded per term (PiX[s][p, (ih, iw, kw7)]) so the
            # kh-peeled multiplies run with step-1 operands (2x bf16 mode).
            # layouts: W/tmp/Y rows are (i, kh, kw) at i*56 + kh*8 + kw with
            # pad column kw=7; the (i,kh) pair merges into one stride-8 dim
            # of 448 (m = 7i + kh), giving 2-dim non-pad views.
            Wv = w_pool.tile([P, EFSP], BF16)
            Wg = wg_pool.tile([P, EFSP], BF16)
            PiX = pix_pool.tile([P, NS * 512], BF16)
            nc.vector.memset(APx(PiX, 7, [[NS * 512, P], [8, NS * 64]]), 0.0)
            for si in range(NS):
                nc.vector.tensor_copy(
                    APx(PiX, si * 512, [[NS * 512, P], [64, 8], [8, 8], [1, 7]]),
                    APx(G9, si * NQ16 + 51, [[G9FS, P], [16, 8], [1, 8], [0, 7]]),
                )

            def term(eng, si, dst):
                for kh in range(7):
                    eng.tensor_tensor(
                        APx(dst, kh * 8, [[EFSP, P], [448, 8], [56, 8], [1, 8]]),
                        APx(PiX, si * 512, [[NS * 512, P], [64, 8], [8, 8], [1, 8]]),
                        APx(G9, si * NQ16 + kh * 16,
                            [[G9FS, P], [16, 8], [1, 8], [1, 8]]),
                        op=mult)

            def flat(t):
                return APx(t, 0, [[EFSP, P], [1, EFSP]])

            term(nc.vector, 0, Wv)
            for si in (1, 2, 3, 4, 5, 6):
                tmpD = tmp_pool.tile([P, EFSP], BF16, tag="tmpd")
                term(nc.vector, si, tmpD)
                nc.vector.tensor_tensor(flat(Wv), flat(Wv), flat(tmpD), op=add)
            term(nc.gpsimd, 7, Wg)
            tmpG = tmp_pool.tile([P, EFSP], BF16, tag="tmpg")
            term(nc.gpsimd, 8, tmpG)
            nc.gpsimd.tensor_tensor(flat(Wg), flat(Wg), flat(tmpG), op=add)
            nc.vector.tensor_tensor(flat(Wv), flat(Wv), flat(Wg), op=add)

            # ---- per-head: attn -> exp -> y -> sum_k -> normalize -> out
            for hd in range(HD):
                E = e_pool.tile([P, EFS + 8], F32)
                nc.scalar.dma_start(
                    APx(E, 0, [[EFS + 8, P], [1, EFS]]),
                    APx(attn_d, (T * HD + hd) * P * EFS, [[EFS, P], [1, EFS]]),
                )
                nc.vector.memset(APx(E, EFS, [[EFS + 8, P], [1, 8]]), 0.0)
                Eb = eb_pool.tile([P, EFSP], BF16)
                nc.scalar.activation(
                    APx(Eb, 0, [[EFSP, P], [8, 448], [1, 8]]),
                    APx(E, 0, [[EFS + 8, P], [7, 448], [1, 8]]),
                    mybir.ActivationFunctionType.Exp)
                Yp = y_pool.tile([P, EFSP], BF16)
                neng = nc.vector if hd < 2 else nc.gpsimd
                nc.vector.tensor_tensor(flat(Yp), flat(Eb), flat(Wv), op=mult)
                Ssum = s_pool.tile([P, NI], F32, tag="ssum")
                Rcp = s_pool.tile([P, NI], F32, tag="rcp")
                nc.vector.tensor_reduce(
                    Ssum[:], APx(Yp, 0, [[EFSP, P], [56, NI], [1, 56]]),
                    axis=mybir.AxisListType.X, op=add)
                nc.vector.tensor_scalar_add(Rcp[:], Ssum[:], 1e-15)
                nc.vector.reciprocal(Rcp[:], Rcp[:])
                # normalize, writing f32 compact into the (now free) E tile
                neng.tensor_tensor(
                    APx(E, 0, [[EFS + 8, P], [49, 64], [7, 7], [1, 7]]),
                    APx(Yp, 0, [[EFSP, P], [56, 64], [8, 7], [1, 7]]),
                    APx(Rcp, 0, [[NI, P], [1, NI], [0, 7], [0, 7]]), op=mult)
                nc.sync.dma_start(
                    APx(out_d, (T * HD + hd) * P * EFS, [[EFS, P], [1, EFS]]),
                    APx(E, 0, [[EFS + 8, P], [1, EFS]]),
                )

    nc.compile()
    return nc


def shard_inputs(attn, sims):
    """Full inputs -> per-core in_maps (list of 8 dicts)."""
    attn = np.ascontiguousarray(attn, dtype=np.float32)
    sims = np.ascontiguousarray(sims, dtype=np.float32)
    in_maps = []
    th = np.arange(5)
    for c in range(N_CORES):
        b, j = divmod(c, 4)
        a = attn[b, :, 64 * j:64 * j + 64]            # (hd, 64, 256, 49)
        a = a.reshape(HD, NT, HBT, 8, NBW, 8, K)
        a = a.transpose(1, 0, 2, 4, 3, 5, 6)          # T, hd, hbl, wb, ih, iw, k
        attn_shard = np.ascontiguousarray(a.reshape(NT, HD, P, EFS))
        gpos = np.arange(64 * j - HALO, 64 * j + BAND + HALO)
        gval = np.clip(gpos, 0, H - 1)
        rows = sims[b, gval]                          # (70, 256, 32, 32)
        sh = (gpos[:, None] // 8) + th[None, :] - 2   # (70, 5)
        valid = (sh >= 0) & (sh < SH)
        shc = np.clip(sh, 0, SH - 1)
        slab = np.take_along_axis(rows, shc[:, None, :, None], axis=2)
        slab = np.where(valid[:, None, :, None], slab, np.float32(0.0))
        import ml_dtypes
        in_maps.append({"attn": attn_shard,
                        "slab": np.ascontiguousarray(slab.astype(ml_dtypes.bfloat16))})
    return in_maps


def unshard_output(results):
    out = np.empty((B, HD, H, W, K), dtype=np.float32)
    for c in range(N_CORES):
        b, j = divmod(c, 4)
        o = results[c]["out"].reshape(NT, HD, HBT, NBW, 8, 8, K)
        o = o.transpose(1, 0, 2, 4, 3, 5, 6)          # hd, T, hbl, ih, wb, iw, k
        out[b, :, 64 * j:64 * j + 64] = o.reshape(HD, BAND, W, K)
    return out


_NC_CACHE = {}


def kernel(attn, sims):
    from concourse.bass_utils import run_bass_kernel_spmd
    if "nc" not in _NC_CACHE:
        _NC_CACHE["nc"] = build_graph()
    nc = _NC_CACHE["nc"]
    in_maps = shard_inputs(attn, sims)
    res = run_bass_kernel_spmd(nc, in_maps, core_ids=list(range(N_CORES)))
    return unshard_output(res.results)


# revision 24
# speedup vs baseline: 1.5209x; 1.2566x over previous
"""Trainium2 Bass kernel for nn_AttnReweight (superpixel-reweighted attention).

Math (per batch b, head hd, pixel (h,w), key k in a 7x7 window):
    w[b,h,w,k] = sum_{s in 3x3 superpixel nbhd} Pi[b,h,w,s] * Pj[b,s,h,w,k]
    out = (w * exp(attn)) / (eps + sum_k w * exp(attn))
(The reference's max-shift cancels in the ratio; attn ~ N(0,1) so exp() is
safe in fp32 without it.)

Sharding: 8 cores = 2 batches x 4 row-bands of 64 rows. Each core gets
  - its attn shard, pre-swizzled to the on-chip (tile, head, block, pixel)
    layout so loads/stores are two maximal contiguous DMAs per (tile, head)
  - a "slab" shard: for each of its 70 rows (64 + 3 halo each side, rows
    clamped at the image border) the 5 superpixel-table rows that any query
    window positioned at that row can touch, zero-masked where the plane
    index falls outside the 32x32 superpixel grid.
All remaining work is on-device and identical on every core (SPMD):
per-pixel 5x5 window extraction, per-block (8x8-pixel) region tiles,
the 9-term superpixel einsum, exp/normalize, and the output writeback.
"""

import sys

sys.path.insert(0, "/opt/trn_rl_repo")

import numpy as np

import concourse.bass as bass
import concourse.tile as tile
from concourse import bacc, mybir
from contextlib import ExitStack

F32 = mybir.dt.float32
BF16 = mybir.dt.bfloat16

# problem geometry (hardcoded per the harness contract)
B, HD, H, W, K = 2, 4, 256, 256, 49
SH = SW = 32
N_CORES = 8
BAND = 64          # pixel rows per core
HALO = 3
NROW = BAND + 2 * HALO          # 70 A rows per core
NT = 2                          # tiles per core (block-row halves)
HBT = 4                         # block-rows per tile
NBW = 32                        # block-cols
P = HBT * NBW                   # 128 partitions (blocks) per tile
NQ = 14 * 14                    # region pixels per block
NI = 64                         # pixels per block
NK = 49
NS = 9
APAD = 75                       # 3 pixels * 25 on each w side
AFS = APAD + 256 * 25 + APAD    # A free size (w-major, 25-patch inner)
G25FS = NQ * 25                 # 4900
NQ16 = 14 * 16                  # padded region row pitch
G9FS = NS * NQ16                # 2016
EFS = NI * NK                   # 3136 (compact i,k)
EFSP = NI * 56                  # 3584 (k padded to 56 for alignment)
WC = 32                         # slab w-chunk
SLABPAD = 64
SLABFS = WC * 160 + 2 * SLABPAD


def APx(t, off, dims):
    return bass.AP(t.tensor, off, [list(d) for d in dims])


def build_graph():
    nc = bacc.Bacc("TRN2", target_bir_lowering=False, debug=False,
                   num_devices=N_CORES)
    attn_d = nc.dram_tensor("attn", [NT, HD, P, EFS], F32, kind="ExternalInput").ap()
    slab_d = nc.dram_tensor("slab", [NROW, W, 5, SW], BF16, kind="ExternalInput").ap()
    out_d = nc.dram_tensor("out", [NT, HD, P, EFS], F32, kind="ExternalOutput").ap()

    mult, add = mybir.AluOpType.mult, mybir.AluOpType.add

    with tile.TileContext(nc) as tc, ExitStack() as ctx:
        slab_pool = ctx.enter_context(tc.tile_pool(name="slab", bufs=2))
        a_pool = ctx.enter_context(tc.tile_pool(name="apool", bufs=1))
        g25_pool = ctx.enter_context(tc.tile_pool(name="g25", bufs=2))
        g9_pool = ctx.enter_context(tc.tile_pool(name="g9", bufs=2))
        pix_pool = ctx.enter_context(tc.tile_pool(name="pix", bufs=2))
        e_pool = ctx.enter_context(tc.tile_pool(name="epool", bufs=2))
        eb_pool = ctx.enter_context(tc.tile_pool(name="ebpool", bufs=2))
        y_pool = ctx.enter_context(tc.tile_pool(name="ypool", bufs=2))
        w_pool = ctx.enter_context(tc.tile_pool(name="wpool", bufs=2))
        tmp_pool = ctx.enter_context(tc.tile_pool(name="tmp", bufs=3))
        wg_pool = ctx.enter_context(tc.tile_pool(name="wgpool", bufs=1))
        s_pool = ctx.enter_context(tc.tile_pool(name="spool", bufs=4))
        d_pool = ctx.enter_context(tc.tile_pool(name="dstage", bufs=1, space="DRAM"))

        A = a_pool.tile([NROW, AFS], BF16)
        Ad = d_pool.tile([NROW, AFS], BF16)
        # zero the w-padding columns once (read by the full-width G25 DMA)
        nc.vector.memset(APx(A, 0, [[AFS, NROW], [1, APAD]]), 0.0)
        nc.vector.memset(APx(A, APAD + 256 * 25, [[AFS, NROW], [1, APAD]]), 0.0)

        # ---- stage 1: slab load + per-pixel 5x5 window extraction into A
        # A[r, 75 + w*25 + th*5 + tw] = slab[r, w, th, (w//8) + tw - 2]
        for c in range(W // WC):
            SB = slab_pool.tile([NROW, SLABFS], BF16)
            nc.vector.memset(APx(SB, 0, [[SLABFS, NROW], [1, SLABPAD]]), 0.0)
            nc.vector.memset(
                APx(SB, SLABPAD + WC * 160, [[SLABFS, NROW], [1, SLABPAD]]), 0.0)
            nc.sync.dma_start(
                APx(SB, SLABPAD, [[SLABFS, NROW], [1, WC * 160]]),
                APx(slab_d, c * WC * 160, [[W * 160, NROW], [1, WC * 160]]),
            )
            nwb = WC // 8
            src = APx(SB, SLABPAD + (c * nwb) - 2,
                      [[SLABFS, NROW], [8 * 160 + 1, nwb], [160, 8], [32, 5], [1, 5]])
            dst = APx(A, APAD + c * WC * 25,
                      [[AFS, NROW], [200, nwb], [25, 8], [5, 5], [1, 5]])
            nc.vector.tensor_copy(dst, src)

        # zero window columns whose superpixel column falls outside [0,32)
        for w0, nw, tc0, ntc in ((0, 8, 0, 2), (8, 8, 0, 1),
                                 (240, 8, 4, 1), (248, 8, 3, 2)):
            nc.vector.memset(
                APx(A, APAD + w0 * 25 + tc0,
                    [[AFS, NROW], [25, nw], [5, 5], [1, ntc]]), 0.0)
        # fill the w-padding with the border pixel's patch, re-expressed in
        # the out-of-range region position's frame (clipped key pixels)
        nc.vector.tensor_copy(
            APx(A, 0 * 25 + 2, [[AFS, NROW], [25, 3], [5, 5], [1, 3]]),
            APx(A, APAD + 0 * 25 + 1, [[AFS, NROW], [0, 3], [5, 5], [1, 3]]),
        )
        nc.vector.tensor_copy(
            APx(A, APAD + 256 * 25 + 0, [[AFS, NROW], [25, 3], [5, 5], [1, 3]]),
            APx(A, APAD + 255 * 25 + 1, [[AFS, NROW], [0, 3], [5, 5], [1, 3]]),
        )
        # stage A to DRAM (SBUF APs cannot express the partition-crossing
        # A -> G25 rearrange on both sides; DRAM APs are flat)
        nc.sync.dma_start(Ad[:], A[:])

        # ---- per-tile processing
        for T in range(NT):
            # G25[p = hbl*32+wb, (qh*14+qw)*25 + t] = A[32T+8hbl+qh, w=8wb+qw-3, t]
            G25 = g25_pool.tile([P, G25FS], BF16)
            for hbl in range(HBT):
                nc.sync.dma_start(
                    APx(G25, hbl * 32 * G25FS,
                        [[G25FS, NBW], [14 * 25, 14], [1, 350]]),
                    APx(Ad, (32 * T + 8 * hbl) * AFS + APAD - 3 * 25,
                        [[200, NBW], [AFS, 14], [1, 350]]),
                )

            # ---- G9: rectangularize per (s, dd); ACT + GpSimd do the copies
            G9 = g9_pool.tile([P, G9FS], BF16)
            nc.gpsimd.memset(
                APx(G9, 14, [[G9FS, P], [16, NS * 14], [1, 2]]), 0.0)
            engs = [nc.scalar, nc.gpsimd]
            ci = 0
            for si in range(NS):
                dh, dw = si // 3 - 1, si % 3 - 1
                for ddh in (-1, 0, 1):
                    for ddw in (-1, 0, 1):
                        qh0, nqh = {(-1): (0, 3), 0: (3, 8), 1: (11, 3)}[ddh]
                        qw0, nqw = {(-1): (0, 3), 0: (3, 8), 1: (11, 3)}[ddw]
                        tcol = (dh - ddh + 2) * 5 + (dw - ddw + 2)
                        src = APx(G25, (qh0 * 14 + qw0) * 25 + tcol,
                                  [[G25FS, P], [14 * 25, nqh], [25, nqw]])
                        dst = APx(G9, si * NQ16 + qh0 * 16 + qw0,
                                  [[G9FS, P], [16, nqh], [1, nqw]])
                        eng = engs[ci % 2]
                        ci += 1
                        if eng is nc.scalar:
                            eng.copy(dst, src)
                        else:
                            eng.tensor_copy(dst, src)

            # ---- einsum: W[p, i, kpad56] = sum_s Pi_s * Pj_s
            # Pi is pre-expanded per term (PiX[s][p, (ih, iw, kw7)]) so the
            # kh-peeled multiplies run with step-1 operands (2x bf16 mode).
            # layouts: W/tmp/Y rows are (i, kh, kw) at i*56 + kh*8 + kw with
            # pad column kw=7; the (i,kh) pair merges into one stride-8 dim
            # of 448 (m = 7i + kh), giving 2-dim non-pad views.
            Wv = w_pool.tile([P, EFSP], BF16)
            Wg = wg_pool.tile([P, EFSP], BF16)
            PiX = pix_pool.tile([P, NS * 512], BF16)
            nc.vector.memset(APx(PiX, 7, [[NS * 512, P], [8, NS * 64]]), 0.0)
            for si in range(NS):
                nc.scalar.copy(
                    APx(PiX, si * 512, [[NS * 512, P], [64, 8], [8, 8], [1, 7]]),
                    APx(G9, si * NQ16 + 51, [[G9FS, P], [16, 8], [1, 8], [0, 7]]),
                )

            def term(eng, si, dst):
                for kh in range(7):
                    eng.tensor_tensor(
                        APx(dst, kh * 8, [[EFSP, P], [448, 8], [56, 8], [1, 8]]),
                        APx(PiX, si * 512, [[NS * 512, P], [64, 8], [8, 8], [1, 8]]),
                        APx(G9, si * NQ16 + kh * 16,
                            [[G9FS, P], [16, 8], [1, 8], [1, 8]]),
                        op=mult)

            def flat(t):
                return APx(t, 0, [[EFSP, P], [1, EFSP]])

            # tree-structured accumulation (shorter bf16 error chains)
            term(nc.vector, 0, Wv)
            t1 = tmp_pool.tile([P, EFSP], BF16, tag="tmpd")
            term(nc.vector, 1, t1)
            nc.vector.tensor_tensor(flat(Wv), flat(Wv), flat(t1), op=add)
            u1 = wg_pool.tile([P, EFSP], BF16, tag="wg")
            t2 = tmp_pool.tile([P, EFSP], BF16, tag="tmpd")
            term(nc.vector, 2, u1)
            term(nc.vector, 3, t2)
            nc.vector.tensor_tensor(flat(u1), flat(u1), flat(t2), op=add)
            nc.vector.tensor_tensor(flat(Wv), flat(Wv), flat(u1), op=add)
            u2 = wg_pool.tile([P, EFSP], BF16, tag="wg2")
            t3 = tmp_pool.tile([P, EFSP], BF16, tag="tmpd")
            term(nc.vector, 4, u2)
            term(nc.vector, 5, t3)
            nc.vector.tensor_tensor(flat(u2), flat(u2), flat(t3), op=add)
            u3 = wg_pool.tile([P, EFSP], BF16, tag="wg3")
            t4 = tmp_pool.tile([P, EFSP], BF16, tag="tmpd")
            term(nc.vector, 6, u3)
            term(nc.vector, 7, t4)
            nc.vector.tensor_tensor(flat(u3), flat(u3), flat(t4), op=add)
            nc.vector.tensor_tensor(flat(u2), flat(u2), flat(u3), op=add)
            t5 = tmp_pool.tile([P, EFSP], BF16, tag="tmpd")
            term(nc.vector, 8, t5)
            nc.vector.tensor_tensor(flat(u2), flat(u2), flat(t5), op=add)
            nc.vector.tensor_tensor(flat(Wv), flat(Wv), flat(u2), op=add)

            # ---- per-head: attn -> exp -> y -> sum_k -> normalize -> out
            for hd in range(HD):
                E = e_pool.tile([P, EFS + 8], F32)
                nc.scalar.dma_start(
                    APx(E, 0, [[EFS + 8, P], [1, EFS]]),
                    APx(attn_d, (T * HD + hd) * P * EFS, [[EFS, P], [1, EFS]]),
                )
                nc.vector.memset(APx(E, EFS, [[EFS + 8, P], [1, 8]]), 0.0)
                Eb = eb_pool.tile([P, EFSP], BF16)
                nc.scalar.activation(
                    APx(Eb, 0, [[EFSP, P], [8, 448], [1, 8]]),
                    APx(E, 0, [[EFS + 8, P], [7, 448], [1, 8]]),
                    mybir.ActivationFunctionType.Exp)
                Yp = y_pool.tile([P, EFSP], BF16)
                neng = nc.gpsimd
                nc.vector.tensor_tensor(flat(Yp), flat(Eb), flat(Wv), op=mult)
                Ssum = s_pool.tile([P, NI], F32, tag="ssum")
                Rcp = s_pool.tile([P, NI], F32, tag="rcp")
                nc.vector.tensor_reduce(
                    Ssum[:], APx(Yp, 0, [[EFSP, P], [56, NI], [1, 56]]),
                    axis=mybir.AxisListType.X, op=add)
                nc.vector.tensor_scalar_add(Rcp[:], Ssum[:], 1e-15)
                nc.vector.reciprocal(Rcp[:], Rcp[:])
                # normalize, writing f32 compact into the (now free) E tile
                neng.tensor_tensor(
                    APx(E, 0, [[EFS + 8, P], [49, 64], [7, 7], [1, 7]]),
                    APx(Yp, 0, [[EFSP, P], [56, 64], [8, 7], [1, 7]]),
                    APx(Rcp, 0, [[NI, P], [1, NI], [0, 7], [0, 7]]), op=mult)
                nc.sync.dma_start(
                    APx(out_d, (T * HD + hd) * P * EFS, [[EFS, P], [1, EFS]]),
                    APx(E, 0, [[EFS + 8, P], [1, EFS]]),
                )

    nc.compile()
    return nc


def shard_inputs(attn, sims):
    """Full inputs -> per-core in_maps (list of 8 dicts)."""
    attn = np.ascontiguousarray(attn, dtype=np.float32)
    sims = np.ascontiguousarray(sims, dtype=np.float32)
    in_maps = []
    th = np.arange(5)
    for c in range(N_CORES):
        b, j = divmod(c, 4)
        a = attn[b, :, 64 * j:64 * j + 64]            # (hd, 64, 256, 49)
        a = a.reshape(HD, NT, HBT, 8, NBW, 8, K)
        a = a.transpose(1, 0, 2, 4, 3, 5, 6)          # T, hd, hbl, wb, ih, iw, k
        attn_shard = np.ascontiguousarray(a.reshape(NT, HD, P, EFS))
        gpos = np.arange(64 * j - HALO, 64 * j + BAND + HALO)
        gval = np.clip(gpos, 0, H - 1)
        rows = sims[b, gval]                          # (70, 256, 32, 32)
        sh = (gpos[:, None] // 8) + th[None, :] - 2   # (70, 5)
        valid = (sh >= 0) & (sh < SH)
        shc = np.clip(sh, 0, SH - 1)
        slab = np.take_along_axis(rows, shc[:, None, :, None], axis=2)
        slab = np.where(valid[:, None, :, None], slab, np.float32(0.0))
        import ml_dtypes
        in_maps.append({"attn": attn_shard,
                        "slab": np.ascontiguousarray(slab.astype(ml_dtypes.bfloat16))})
    return in_maps


def unshard_output(results):
    out = np.empty((B, HD, H, W, K), dtype=np.float32)
    for c in range(N_CORES):
        b, j = divmod(c, 4)
        o = results[c]["out"].reshape(NT, HD, HBT, NBW, 8, 8, K)
        o = o.transpose(1, 0, 2, 4, 3, 5, 6)          # hd, T, hbl, ih, wb, iw, k
        out[b, :, 64 * j:64 * j + 64] = o.reshape(HD, BAND, W, K)
    return out


_NC_CACHE = {}


def kernel(attn, sims):
    from concourse.bass_utils import run_bass_kernel_spmd
    if "nc" not in _NC_CACHE:
        _NC_CACHE["nc"] = build_graph()
    nc = _NC_CACHE["nc"]
    in_maps = shard_inputs(attn, sims)
    res = run_bass_kernel_spmd(nc, in_maps, core_ids=list(range(N_CORES)))
    return unshard_output(res.results)


# revision 26
# speedup vs baseline: 1.5440x; 1.0152x over previous
"""Trainium2 Bass kernel for nn_AttnReweight (superpixel-reweighted attention).

Math (per batch b, head hd, pixel (h,w), key k in a 7x7 window):
    w[b,h,w,k] = sum_{s in 3x3 superpixel nbhd} Pi[b,h,w,s] * Pj[b,s,h,w,k]
    out = (w * exp(attn)) / (eps + sum_k w * exp(attn))
(The reference's max-shift cancels in the ratio; attn ~ N(0,1) so exp() is
safe in fp32 without it.)

Sharding: 8 cores = 2 batches x 4 row-bands of 64 rows. Each core gets
  - its attn shard, pre-swizzled to the on-chip (tile, head, block, pixel)
    layout so loads/stores are two maximal contiguous DMAs per (tile, head)
  - a "slab" shard: for each of its 70 rows (64 + 3 halo each side, rows
    clamped at the image border) the 5 superpixel-table rows that any query
    window positioned at that row can touch, zero-masked where the plane
    index falls outside the 32x32 superpixel grid.
All remaining work is on-device and identical on every core (SPMD):
per-pixel 5x5 window extraction, per-block (8x8-pixel) region tiles,
the 9-term superpixel einsum, exp/normalize, and the output writeback.
"""

import sys

sys.path.insert(0, "/opt/trn_rl_repo")

import numpy as np

import concourse.bass as bass
import concourse.tile as tile
from concourse import bacc, mybir
from contextlib import ExitStack

F32 = mybir.dt.float32
BF16 = mybir.dt.bfloat16

# problem geometry (hardcoded per the harness contract)
B, HD, H, W, K = 2, 4, 256, 256, 49
SH = SW = 32
N_CORES = 8
BAND = 64          # pixel rows per core
HALO = 3
NROW = BAND + 2 * HALO          # 70 A rows per core
NT = 2                          # tiles per core (block-row halves)
HBT = 4                         # block-rows per tile
NBW = 32                        # block-cols
P = HBT * NBW                   # 128 partitions (blocks) per tile
NQ = 14 * 14                    # region pixels per block
NI = 64                         # pixels per block
NK = 49
NS = 9
APAD = 75                       # 3 pixels * 25 on each w side
AFS = APAD + 256 * 25 + APAD    # A free size (w-major, 25-patch inner)
G25FS = NQ * 25                 # 4900
NQ16 = 14 * 16                  # padded region row pitch
G9FS = NS * NQ16                # 2016
EFS = NI * NK                   # 3136 (compact i,k)
EFSP = NI * 56                  # 3584 (k padded to 56 for alignment)
WC = 32                         # slab w-chunk
SLABPAD = 64
SLABFS = WC * 160 + 2 * SLABPAD


def APx(t, off, dims):
    return bass.AP(t.tensor, off, [list(d) for d in dims])


def build_graph():
    nc = bacc.Bacc("TRN2", target_bir_lowering=False, debug=False,
                   num_devices=N_CORES)
    attn_d = nc.dram_tensor("attn", [NT, HD, P, EFS], F32, kind="ExternalInput").ap()
    slab_d = nc.dram_tensor("slab", [NROW, W, 5, SW], BF16, kind="ExternalInput").ap()
    out_d = nc.dram_tensor("out", [NT, HD, P, EFS], F32, kind="ExternalOutput").ap()

    mult, add = mybir.AluOpType.mult, mybir.AluOpType.add

    with tile.TileContext(nc) as tc, ExitStack() as ctx:
        slab_pool = ctx.enter_context(tc.tile_pool(name="slab", bufs=2))
        a_pool = ctx.enter_context(tc.tile_pool(name="apool", bufs=1))
        g25_pool = ctx.enter_context(tc.tile_pool(name="g25", bufs=2))
        g9_pool = ctx.enter_context(tc.tile_pool(name="g9", bufs=2))
        pix_pool = ctx.enter_context(tc.tile_pool(name="pix", bufs=2))
        e_pool = ctx.enter_context(tc.tile_pool(name="epool", bufs=2))
        eb_pool = ctx.enter_context(tc.tile_pool(name="ebpool", bufs=2))
        y_pool = ctx.enter_context(tc.tile_pool(name="ypool", bufs=3))
        w_pool = ctx.enter_context(tc.tile_pool(name="wpool", bufs=2))
        tmp_pool = ctx.enter_context(tc.tile_pool(name="tmp", bufs=3))
        wg_pool = ctx.enter_context(tc.tile_pool(name="wgpool", bufs=1))
        s_pool = ctx.enter_context(tc.tile_pool(name="spool", bufs=4))
        d_pool = ctx.enter_context(tc.tile_pool(name="dstage", bufs=1, space="DRAM"))

        A = a_pool.tile([NROW, AFS], BF16)
        Ad = d_pool.tile([NROW, AFS], BF16)
        # zero the w-padding columns once (read by the full-width G25 DMA)
        nc.vector.memset(APx(A, 0, [[AFS, NROW], [1, APAD]]), 0.0)
        nc.vector.memset(APx(A, APAD + 256 * 25, [[AFS, NROW], [1, APAD]]), 0.0)

        # ---- stage 1: slab load + per-pixel 5x5 window extraction into A
        # A[r, 75 + w*25 + th*5 + tw] = slab[r, w, th, (w//8) + tw - 2]
        for c in range(W // WC):
            SB = slab_pool.tile([NROW, SLABFS], BF16)
            nc.vector.memset(APx(SB, 0, [[SLABFS, NROW], [1, SLABPAD]]), 0.0)
            nc.vector.memset(
                APx(SB, SLABPAD + WC * 160, [[SLABFS, NROW], [1, SLABPAD]]), 0.0)
            nc.sync.dma_start(
                APx(SB, SLABPAD, [[SLABFS, NROW], [1, WC * 160]]),
                APx(slab_d, c * WC * 160, [[W * 160, NROW], [1, WC * 160]]),
            )
            nwb = WC // 8
            src = APx(SB, SLABPAD + (c * nwb) - 2,
                      [[SLABFS, NROW], [8 * 160 + 1, nwb], [160, 8], [32, 5], [1, 5]])
            dst = APx(A, APAD + c * WC * 25,
                      [[AFS, NROW], [200, nwb], [25, 8], [5, 5], [1, 5]])
            nc.vector.tensor_copy(dst, src)

        # zero window columns whose superpixel column falls outside [0,32)
        for w0, nw, tc0, ntc in ((0, 8, 0, 2), (8, 8, 0, 1),
                                 (240, 8, 4, 1), (248, 8, 3, 2)):
            nc.vector.memset(
                APx(A, APAD + w0 * 25 + tc0,
                    [[AFS, NROW], [25, nw], [5, 5], [1, ntc]]), 0.0)
        # fill the w-padding with the border pixel's patch, re-expressed in
        # the out-of-range region position's frame (clipped key pixels)
        nc.vector.tensor_copy(
            APx(A, 0 * 25 + 2, [[AFS, NROW], [25, 3], [5, 5], [1, 3]]),
            APx(A, APAD + 0 * 25 + 1, [[AFS, NROW], [0, 3], [5, 5], [1, 3]]),
        )
        nc.vector.tensor_copy(
            APx(A, APAD + 256 * 25 + 0, [[AFS, NROW], [25, 3], [5, 5], [1, 3]]),
            APx(A, APAD + 255 * 25 + 1, [[AFS, NROW], [0, 3], [5, 5], [1, 3]]),
        )
        # stage A to DRAM (SBUF APs cannot express the partition-crossing
        # A -> G25 rearrange on both sides; DRAM APs are flat)
        nc.sync.dma_start(Ad[:], A[:])

        # ---- per-tile processing
        for T in range(NT):
            # G25[p = hbl*32+wb, (qh*14+qw)*25 + t] = A[32T+8hbl+qh, w=8wb+qw-3, t]
            G25 = g25_pool.tile([P, G25FS], BF16)
            for hbl in range(HBT):
                nc.sync.dma_start(
                    APx(G25, hbl * 32 * G25FS,
                        [[G25FS, NBW], [14 * 25, 14], [1, 350]]),
                    APx(Ad, (32 * T + 8 * hbl) * AFS + APAD - 3 * 25,
                        [[200, NBW], [AFS, 14], [1, 350]]),
                )

            # ---- G9: rectangularize per (s, dd); ACT + GpSimd do the copies
            G9 = g9_pool.tile([P, G9FS], BF16)
            nc.gpsimd.memset(
                APx(G9, 14, [[G9FS, P], [16, NS * 14], [1, 2]]), 0.0)
            engs = [nc.scalar, nc.gpsimd]
            ci = 0
            for si in range(NS):
                dh, dw = si // 3 - 1, si % 3 - 1
                for ddh in (-1, 0, 1):
                    for ddw in (-1, 0, 1):
                        qh0, nqh = {(-1): (0, 3), 0: (3, 8), 1: (11, 3)}[ddh]
                        qw0, nqw = {(-1): (0, 3), 0: (3, 8), 1: (11, 3)}[ddw]
                        tcol = (dh - ddh + 2) * 5 + (dw - ddw + 2)
                        src = APx(G25, (qh0 * 14 + qw0) * 25 + tcol,
                                  [[G25FS, P], [14 * 25, nqh], [25, nqw]])
                        dst = APx(G9, si * NQ16 + qh0 * 16 + qw0,
                                  [[G9FS, P], [16, nqh], [1, nqw]])
                        eng = engs[ci % 2]
                        ci += 1
                        if eng is nc.scalar:
                            eng.copy(dst, src)
                        else:
                            eng.tensor_copy(dst, src)

            # ---- einsum: W[p, i, kpad56] = sum_s Pi_s * Pj_s
            # Pi is pre-expanded per term (PiX[s][p, (ih, iw, kw7)]) so the
            # kh-peeled multiplies run with step-1 operands (2x bf16 mode).
            # layouts: W/tmp/Y rows are (i, kh, kw) at i*56 + kh*8 + kw with
            # pad column kw=7; the (i,kh) pair merges into one stride-8 dim
            # of 448 (m = 7i + kh), giving 2-dim non-pad views.
            Wv = w_pool.tile([P, EFSP], BF16)
            Wg = wg_pool.tile([P, EFSP], BF16)
            PiX = pix_pool.tile([P, NS * 512], BF16)
            nc.vector.memset(APx(PiX, 7, [[NS * 512, P], [8, NS * 64]]), 0.0)
            for si in range(NS):
                nc.scalar.copy(
                    APx(PiX, si * 512, [[NS * 512, P], [64, 8], [8, 8], [1, 7]]),
                    APx(G9, si * NQ16 + 51, [[G9FS, P], [16, 8], [1, 8], [0, 7]]),
                )

            def term(eng, si, dst):
                for kh in range(7):
                    eng.tensor_tensor(
                        APx(dst, kh * 8, [[EFSP, P], [448, 8], [56, 8], [1, 8]]),
                        APx(PiX, si * 512, [[NS * 512, P], [64, 8], [8, 8], [1, 8]]),
                        APx(G9, si * NQ16 + kh * 16,
                            [[G9FS, P], [16, 8], [1, 8], [1, 8]]),
                        op=mult)

            def flat(t):
                return APx(t, 0, [[EFSP, P], [1, EFSP]])

            # tree-structured accumulation (shorter bf16 error chains)
            term(nc.vector, 0, Wv)
            t1 = tmp_pool.tile([P, EFSP], BF16, tag="tmpd")
            term(nc.vector, 1, t1)
            nc.vector.tensor_tensor(flat(Wv), flat(Wv), flat(t1), op=add)
            u1 = wg_pool.tile([P, EFSP], BF16, tag="wg")
            t2 = tmp_pool.tile([P, EFSP], BF16, tag="tmpd")
            term(nc.vector, 2, u1)
            term(nc.vector, 3, t2)
            nc.vector.tensor_tensor(flat(u1), flat(u1), flat(t2), op=add)
            nc.vector.tensor_tensor(flat(Wv), flat(Wv), flat(u1), op=add)
            u2 = wg_pool.tile([P, EFSP], BF16, tag="wg2")
            t3 = tmp_pool.tile([P, EFSP], BF16, tag="tmpd")
            term(nc.vector, 4, u2)
            term(nc.vector, 5, t3)
            nc.vector.tensor_tensor(flat(u2), flat(u2), flat(t3), op=add)
            u3 = wg_pool.tile([P, EFSP], BF16, tag="wg3")
            t4 = tmp_pool.tile([P, EFSP], BF16, tag="tmpd")
            term(nc.vector, 6, u3)
            term(nc.vector, 7, t4)
            nc.vector.tensor_tensor(flat(u3), flat(u3), flat(t4), op=add)
            nc.vector.tensor_tensor(flat(u2), flat(u2), flat(u3), op=add)
            t5 = tmp_pool.tile([P, EFSP], BF16, tag="tmpd")
            term(nc.vector, 8, t5)
            nc.vector.tensor_tensor(flat(u2), flat(u2), flat(t5), op=add)
            nc.vector.tensor_tensor(flat(Wv), flat(Wv), flat(u2), op=add)

            # ---- per-head: attn -> exp -> y -> sum_k -> normalize -> out
            for hd in range(HD):
                E = e_pool.tile([P, EFS + 8], F32)
                nc.scalar.dma_start(
                    APx(E, 0, [[EFS + 8, P], [1, EFS]]),
                    APx(attn_d, (T * HD + hd) * P * EFS, [[EFS, P], [1, EFS]]),
                )
                nc.vector.memset(APx(E, EFS, [[EFS + 8, P], [1, 8]]), 0.0)
                Eb = eb_pool.tile([P, EFSP], BF16)
                nc.scalar.activation(
                    APx(Eb, 0, [[EFSP, P], [8, 448], [1, 8]]),
                    APx(E, 0, [[EFS + 8, P], [7, 448], [1, 8]]),
                    mybir.ActivationFunctionType.Exp)
                Yp = y_pool.tile([P, EFSP], BF16)
                neng = nc.gpsimd if (T == 0 or hd < 2) else nc.vector
                nc.vector.tensor_tensor(flat(Yp), flat(Eb), flat(Wv), op=mult)
                Ssum = s_pool.tile([P, NI], F32, tag="ssum")
                Rcp = s_pool.tile([P, NI], F32, tag="rcp")
                nc.vector.tensor_reduce(
                    Ssum[:], APx(Yp, 0, [[EFSP, P], [56, NI], [1, 56]]),
                    axis=mybir.AxisListType.X, op=add)
                nc.vector.tensor_scalar_add(Rcp[:], Ssum[:], 1e-15)
                nc.vector.reciprocal(Rcp[:], Rcp[:])
                # normalize, writing f32 compact into the (now free) E tile
                neng.tensor_tensor(
                    APx(E, 0, [[EFS + 8, P], [49, 64], [7, 7], [1, 7]]),
                    APx(Yp, 0, [[EFSP, P], [56, 64], [8, 7], [1, 7]]),
                    APx(Rcp, 0, [[NI, P], [1, NI], [0, 7], [0, 7]]), op=mult)
                nc.sync.dma_start(
                    APx(out_d, (T * HD + hd) * P * EFS, [[EFS, P], [1, EFS]]),
                    APx(E, 0, [[EFS + 8, P], [1, EFS]]),
                )

    nc.compile()
    return nc


def shard_inputs(attn, sims):
    """Full inputs -> per-core in_maps (list of 8 dicts)."""
    attn = np.ascontiguousarray(attn, dtype=np.float32)
    sims = np.ascontiguousarray(sims, dtype=np.float32)
    in_maps = []
    th = np.arange(5)
    for c in range(N_CORES):
        b, j = divmod(c, 4)
        a = attn[b, :, 64 * j:64 * j + 64]            # (hd, 64, 256, 49)
        a = a.reshape(HD, NT, HBT, 8, NBW, 8, K)
        a = a.transpose(1, 0, 2, 4, 3, 5, 6)          # T, hd, hbl, wb, ih, iw, k
        attn_shard = np.ascontiguousarray(a.reshape(NT, HD, P, EFS))
        gpos = np.arange(64 * j - HALO, 64 * j + BAND + HALO)
        gval = np.clip(gpos, 0, H - 1)
        rows = sims[b, gval]                          # (70, 256, 32, 32)
        sh = (gpos[:, None] // 8) + th[None, :] - 2   # (70, 5)
        valid = (sh >= 0) & (sh < SH)
        shc = np.clip(sh, 0, SH - 1)
        slab = np.take_along_axis(rows, shc[:, None, :, None], axis=2)
        slab = np.where(valid[:, None, :, None], slab, np.float32(0.0))
        import ml_dtypes
        in_maps.append({"attn": attn_shard,
                        "slab": np.ascontiguousarray(slab.astype(ml_dtypes.bfloat16))})
    return in_maps


def unshard_output(results):
    out = np.empty((B, HD, H, W, K), dtype=np.float32)
    for c in range(N_CORES):
        b, j = divmod(c, 4)
        o = results[c]["out"].reshape(NT, HD, HBT, NBW, 8, 8, K)
        o = o.transpose(1, 0, 2, 4, 3, 5, 6)          # hd, T, hbl, ih, wb, iw, k
        out[b, :, 64 * j:64 * j + 64] = o.reshape(HD, BAND, W, K)
    return out


_NC_CACHE = {}


def kernel(attn, sims):
    from concourse.bass_utils import run_bass_kernel_spmd
    if "nc" not in _NC_CACHE:
        _NC_CACHE["nc"] = build_graph()
    nc = _NC_CACHE["nc"]
    in_maps = shard_inputs(attn, sims)
    res = run_bass_kernel_spmd(nc, in_maps, core_ids=list(range(N_CORES)))
    return unshard_output(res.results)
